# revision 37
# baseline (speedup 1.0000x reference)
"""Trainium2 Bass kernel for nn_Clusterer loss (Concrete-mixture clustering loss).

Strategy (data-parallel over N across 8 cores, per sharding hint):
  - Minimal wire traffic through the PJRT tunnel (~12.3MB vs 80MB raw fp32):
      z   -> int4 (two values per byte, rows 128 apart share a byte so both
             nibbles land on the same SBUF partition), fixed scale, NATURAL
             row layout; unpacked on device with DVE bitwise and/shift and
             dequantized by ACT Copy (scale=L, bias=-8L)
      met -> int8 transposed [16, NS], per-dim adaptive scales
    The int4 quantization bias of the ln-sum terms is removed by an exact
    sampled correction: stat(z) - stat(dequant(z)) on a deterministic strided
    row subsample in f64 on the host, scaled to N - computed while the
    transfer streams, so it costs no wall time.
  - logN per 128-row tile by four accumulated fp16 matmuls:
      mm1: met16 x w;  mm2/mm3: xsq(=ACT Square of met16) x a_hi / a_lo;
      mm4: ones(2) x [cck_hi; cck_lo]
    (x2 is built on device; hi/lo split of a_k kills fp16 systematic error;
     int8 quantization E[d^2] bias is compensated in cck / const0 on host.)
  - v = logN + z; row logsumexp on DVE/ACT.  Concrete-prior row sums
    (sum e^z, sum pi e^{-tau z}, sum z) are free-axis reductions on the
    natural-layout z tile - z is shipped exactly once, never transposed.
  - Per-row total = max_v + ln su + 63 ln sz - 64 ln st - 1.1 s1 summed on
    device to [128, 1] per core; final f64 reduction + tiny parameter
    losses on host (overlapped with the device call).
  - Dispatch through a cached jit(shard_map(...)) built once per process:
    no per-call retracing, no concat copies (int8 z global array IS the
    axis-0-sharded layout).
"""

import math

import numpy as np

N, D, K = 262144, 16, 64
NCORES = 8
NS = N // NCORES          # rows per core = 32768
NG = NS // 128            # 128-row groups per core = 256
G_SC = 16                 # groups per super-chunk
N_SC = NG // G_SC         # super-chunks = 16
FD_SC = G_SC * 64         # free dim per SC = 1024
TAU = 0.1
LOG2PI = math.log(2.0 * math.pi)
ZLIM = 6.8                # fixed |z| quantization range
L4 = ZLIM / 7.49          # int4 step: codes rint(z/L4) in [-7, 7], biased +8
MSAMP = 8192              # rows in the sampled quantization-bias correction
NRHS = 50                 # rhs rows: 16 w + 16 a_hi + 16 a_lo + 2 cck hi/lo

_cache = {}


def _build_program():
    import concourse.bacc as bacc
    import concourse.mybir as mybir
    import concourse.tile as tile

    fp16 = mybir.dt.float16
    fp32 = mybir.dt.float32
    int8 = mybir.dt.int8
    uint8 = mybir.dt.uint8
    AF = mybir.ActivationFunctionType
    ALU = mybir.AluOpType
    AX = mybir.AxisListType

    nc = bacc.Bacc("TRN2", target_bir_lowering=False, debug=False,
                   num_devices=NCORES)

    mq = nc.dram_tensor("mq", [16, NS], int8, kind="ExternalInput").ap()
    # packed z split in two args: staging of the later args pipelines behind
    # the wire streaming of the earlier ones (finer args measured faster)
    zq4a = nc.dram_tensor("zq4a", [NS // 4, 64], uint8,
                          kind="ExternalInput").ap()
    zq4b = nc.dram_tensor("zq4b", [NS // 4, 64], uint8,
                          kind="ExternalInput").ap()
    rhsv = nc.dram_tensor("rhsv", [NRHS, 64], fp16, kind="ExternalInput").ap()
    pivec = nc.dram_tensor("pivec", [128, 64], fp32, kind="ExternalInput").ap()
    mscale = nc.dram_tensor("mscale", [16, 1], fp32, kind="ExternalInput").ap()
    out = nc.dram_tensor("out", [128, 1], fp32, kind="ExternalOutput").ap()

    # packed z: byte (s, c, p, k) holds z rows (s*16 + 2c)*128+p (lo nibble)
    # and (s*16 + 2c+1)*128+p (hi nibble); view as [p, s, c, k]
    zq_ra = zq4a.rearrange("(s c p) k -> p s c k",
                           s=N_SC // 2, c=G_SC // 2, p=128)
    zq_rb = zq4b.rearrange("(s c p) k -> p s c k",
                           s=N_SC // 2, c=G_SC // 2, p=128)

    with tile.TileContext(nc) as tc:
        with (
            tc.tile_pool(name="const", bufs=1) as constp,
            tc.tile_pool(name="stats", bufs=1) as statp,
            tc.tile_pool(name="mq", bufs=3) as mqp,
            tc.tile_pool(name="pkd", bufs=2) as pkdp,
            tc.tile_pool(name="xsq", bufs=2) as xsqp,
            tc.tile_pool(name="zqp", bufs=3) as zqp,
            tc.tile_pool(name="z16p", bufs=2) as z16p,
            tc.tile_pool(name="vp", bufs=2) as vp,
            tc.tile_pool(name="scr", bufs=2) as scrp,
            tc.tile_pool(name="ep", bufs=1) as epp,
            tc.tile_pool(name="ps", bufs=2, space="PSUM") as psp,
        ):
            rhsA = constp.tile([16, 64], fp16, tag="rhsA")
            nc.sync.dma_start(rhsA[:], rhsv[0:16, :])
            rhsB = constp.tile([16, 64], fp16, tag="rhsB")
            nc.sync.dma_start(rhsB[:], rhsv[16:32, :])
            rhsC = constp.tile([16, 64], fp16, tag="rhsC")
            nc.sync.dma_start(rhsC[:], rhsv[32:48, :])
            rhsD = constp.tile([2, 64], fp16, tag="rhsD")
            nc.sync.dma_start(rhsD[:], rhsv[48:50, :])
            ones2 = constp.tile([2, 128], fp16, tag="ones2")
            nc.vector.memset(ones2[:], 1.0)
            pi_s = constp.tile([128, 64], fp32, tag="pis")
            nc.sync.dma_start(pi_s[:], pivec[:])
            msc = constp.tile([16, 1], fp32, tag="msc")
            nc.sync.dma_start(msc[:], mscale[:])
            pi_t = constp.tile([128, FD_SC], fp32, tag="pit")
            for i in range(G_SC):
                nc.vector.tensor_copy(pi_t[:, i * 64:(i + 1) * 64], pi_s[:])

            mu_all = statp.tile([128, NG], fp32, tag="mu_all")
            su_all = statp.tile([128, NG], fp32, tag="su_all")
            sz_all = statp.tile([128, NG], fp32, tag="sz_all")
            st_all = statp.tile([128, NG], fp32, tag="st_all")
            s1_all = statp.tile([128, NG], fp32, tag="s1_all")

            for sc in range(N_SC):
                sl = slice(sc * G_SC, (sc + 1) * G_SC)
                fsl = slice(sc * G_SC * 128, (sc + 1) * G_SC * 128)

                mq_t = mqp.tile([16, G_SC * 128], int8, tag="mq")
                nc.sync.dma_start(mq_t[:], mq[:, fsl])
                pkd = pkdp.tile([16, G_SC * 128], fp16, tag="pkd")
                nc.scalar.activation(pkd[:], mq_t[:], AF.Copy,
                                     scale=msc[:, 0:1])
                xsq = xsqp.tile([16, G_SC * 128], fp16, tag="xsq")
                nc.scalar.activation(xsq[:], pkd[:], AF.Square)

                zp_t = zqp.tile([128, FD_SC // 2], uint8, tag="zp")
                zq_r = zq_ra if sc < N_SC // 2 else zq_rb
                sch = sc % (N_SC // 2)
                nc.sync.dma_start(
                    zp_t[:].rearrange("p (o c k) -> p o c k", o=1, k=64),
                    zq_r[:, sch:sch + 1, :, :])
                ulo = zqp.tile([128, FD_SC // 2], uint8, tag="ulo")
                nc.vector.tensor_scalar(ulo[:], zp_t[:], 15, None,
                                        op0=ALU.bitwise_and)
                uhi = zqp.tile([128, FD_SC // 2], uint8, tag="uhi")
                nc.vector.tensor_scalar(uhi[:], zp_t[:], 4, None,
                                        op0=ALU.logical_shift_right)
                z16 = z16p.tile([128, FD_SC], fp16, tag="z16")
                z16v = z16[:].rearrange("p (c t k) -> p c t k", t=2, k=64)
                nc.scalar.activation(
                    z16v[:, :, 0, :],
                    ulo[:].rearrange("p (c k) -> p c k", k=64),
                    AF.Copy, scale=L4, bias=-8.0 * L4)
                nc.scalar.activation(
                    z16v[:, :, 1, :],
                    uhi[:].rearrange("p (c k) -> p c k", k=64),
                    AF.Copy, scale=L4, bias=-8.0 * L4)

                ps = psp.tile([128, FD_SC], fp32, tag="v")
                for g in range(G_SC):
                    gs = slice(g * 128, (g + 1) * 128)
                    nc.tensor.matmul(ps[:, g * 64:(g + 1) * 64],
                                     lhsT=pkd[:, gs], rhs=rhsA[:],
                                     start=True, stop=False)
                    nc.tensor.matmul(ps[:, g * 64:(g + 1) * 64],
                                     lhsT=xsq[:, gs], rhs=rhsB[:],
                                     start=False, stop=False)
                    nc.tensor.matmul(ps[:, g * 64:(g + 1) * 64],
                                     lhsT=xsq[:, gs], rhs=rhsC[:],
                                     start=False, stop=False)
                    nc.tensor.matmul(ps[:, g * 64:(g + 1) * 64],
                                     lhsT=ones2[:], rhs=rhsD[:],
                                     start=False, stop=True)

                v = vp.tile([128, FD_SC], fp32, tag="vv")
                nc.vector.tensor_add(v[:], ps[:], z16[:])
                v3 = v[:].rearrange("p (g k) -> p g k", k=64)
                mu_sl = mu_all[:, sl]
                nc.vector.reduce_max(mu_sl, v3, axis=AX.X)
                vs = scrp.tile([128, FD_SC], fp32, tag="vs")
                mu_b = mu_sl.broadcast_to([128, G_SC, 64])
                nc.vector.scalar_tensor_tensor(
                    vs[:].rearrange("p (g k) -> p g k", k=64),
                    in0=v3, scalar=1.0, in1=mu_b,
                    op0=ALU.mult, op1=ALU.subtract)
                eu = scrp.tile([128, FD_SC], fp16, tag="eu")
                nc.scalar.activation(eu[:], vs[:], AF.Exp)
                nc.vector.reduce_sum(
                    su_all[:, sl],
                    eu[:].rearrange("p (g k) -> p g k", k=64), axis=AX.X)

                e1 = scrp.tile([128, FD_SC], fp16, tag="e1")
                nc.scalar.activation(e1[:], z16[:], AF.Exp)
                nc.vector.reduce_sum(
                    sz_all[:, sl],
                    e1[:].rearrange("p (g k) -> p g k", k=64), axis=AX.X)

                e2 = scrp.tile([128, FD_SC], fp16, tag="e2")
                nc.scalar.activation(e2[:], z16[:], AF.Exp, scale=-TAU)
                w2 = scrp.tile([128, FD_SC], fp32, tag="w2")
                nc.vector.tensor_mul(w2[:], e2[:], pi_t[:])
                nc.vector.reduce_sum(
                    st_all[:, sl],
                    w2[:].rearrange("p (g k) -> p g k", k=64), axis=AX.X)

                nc.vector.reduce_sum(
                    s1_all[:, sl],
                    z16[:].rearrange("p (g k) -> p g k", k=64), axis=AX.X)

            # ---- epilogue: mu + ln su + 63 ln sz - 64 ln st - 1.1 s1 ----
            lnsu = epp.tile([128, NG], fp32, tag="lnsu")
            nc.scalar.activation(lnsu[:], su_all[:], AF.Ln)
            lnsz = epp.tile([128, NG], fp32, tag="lnsz")
            nc.scalar.activation(lnsz[:], sz_all[:], AF.Ln)
            lnst = epp.tile([128, NG], fp32, tag="lnst")
            nc.scalar.activation(lnst[:], st_all[:], AF.Ln)
            acc = epp.tile([128, NG], fp32, tag="acc")
            nc.vector.tensor_add(acc[:], mu_all[:], lnsu[:])
            acc2 = epp.tile([128, NG], fp32, tag="acc2")
            nc.vector.scalar_tensor_tensor(
                acc2[:], in0=lnsz[:], scalar=63.0, in1=acc[:],
                op0=ALU.mult, op1=ALU.add)
            acc3 = epp.tile([128, NG], fp32, tag="acc3")
            nc.vector.scalar_tensor_tensor(
                acc3[:], in0=lnst[:], scalar=-64.0, in1=acc2[:],
                op0=ALU.mult, op1=ALU.add)
            acc4 = epp.tile([128, NG], fp32, tag="acc4")
            nc.vector.scalar_tensor_tensor(
                acc4[:], in0=s1_all[:], scalar=-1.1, in1=acc3[:],
                op0=ALU.mult, op1=ALU.add)
            out_t = epp.tile([128, 1], fp32, tag="outt")
            nc.vector.reduce_sum(out_t[:], acc4[:], axis=AX.X)
            nc.sync.dma_start(out[:], out_t[:])

    nc.compile()
    return nc


def _make_runner(nc):
    """Cached jit(shard_map(...)) dispatcher; built once, reused every call."""
    import jax
    from jax.experimental.shard_map import shard_map
    from jax.sharding import Mesh, PartitionSpec

    import concourse.mybir as mybir
    from concourse.bass2jax import (_bass_exec_p, install_neuronx_cc_hook,
                                    partition_id_tensor)

    install_neuronx_cc_hook()

    partition_name = (nc.partition_id_tensor.name
                      if nc.partition_id_tensor else None)
    in_names, out_names, out_avals, zero_shapes = [], [], [], []
    for alloc in nc.m.functions[0].allocations:
        if not isinstance(alloc, mybir.MemoryLocationSet):
            continue
        name = alloc.memorylocations[0].name
        if alloc.kind == "ExternalInput":
            if name != partition_name:
                in_names.append(name)
        elif alloc.kind == "ExternalOutput":
            shape = tuple(alloc.tensor_shape)
            dtype = mybir.dt.np(alloc.dtype)
            out_names.append(name)
            out_avals.append(jax.core.ShapedArray(shape, dtype))
            zero_shapes.append((shape, dtype))
    n_params = len(in_names)
    all_names = list(in_names) + list(out_names)
    if partition_name is not None:
        all_names.append(partition_name)
    donate = tuple(range(n_params, n_params + len(out_names)))

    def _body(*args):
        operands = list(args)
        if partition_name is not None:
            operands.append(partition_id_tensor())
        outs = _bass_exec_p.bind(
            *operands,
            out_avals=tuple(out_avals),
            in_names=tuple(all_names),
            out_names=tuple(out_names),
            lowering_input_output_aliases=(),
            sim_require_finite=True,
            sim_require_nnan=True,
            nc=nc,
        )
        return tuple(outs)

    devices = jax.devices()[:NCORES]
    assert len(devices) == NCORES
    mesh = Mesh(np.asarray(devices), ("core",))
    in_specs = (PartitionSpec("core"),) * (n_params + len(out_names))
    out_specs = (PartitionSpec("core"),) * len(out_names)
    sharded = jax.jit(
        shard_map(_body, mesh=mesh, in_specs=in_specs, out_specs=out_specs,
                  check_rep=False),
        donate_argnums=donate, keep_unused=True)

    def run(globals_map):
        """Dispatch; returns unforced jax outputs keyed by name."""
        ins = [globals_map[n] for n in in_names]
        zeros = [np.zeros((NCORES * s[0], *s[1:]), d) for s, d in zero_shapes]
        outs = sharded(*ins, *zeros)
        return {name: outs[i] for i, name in enumerate(out_names)}

    return run


def _prep_consts(mu, pi, r, met_l2):
    """K-sized constants in f64 -> fp16 hi/lo packed rhs + pi softmax.

    met_l2 = sum_f LSB_f^2 of the met quantizer (for E[d^2] bias comp).
    """
    f64 = np.float64
    mu64 = mu.astype(f64)
    r64 = r.astype(f64)
    pi64 = pi.astype(f64)

    a = -0.5 * np.exp(-r64)                       # [K]
    mu2 = (mu64 ** 2).sum(1)                      # [K]
    ck = -0.5 * D * (r64 + LOG2PI)                # [K]
    cck = a * mu2 + ck - a * (met_l2 / 12.0)      # [K], with quant-bias comp
    m = pi64.max()
    lnpi64 = pi64 - (m + np.log(np.exp(pi64 - m).sum()))
    pisoft = np.exp(lnpi64)

    rhsv = np.zeros((NRHS, 64), np.float16)
    rhsv[0:16, :] = (-2.0 * a[None, :] * mu64.T).astype(np.float16)
    a_hi = a.astype(np.float16)
    a_lo = (a - a_hi.astype(f64)).astype(np.float16)
    rhsv[16:32, :] = a_hi[None, :]
    rhsv[32:48, :] = a_lo[None, :]
    cck_hi = cck.astype(np.float16)
    rhsv[48, :] = cck_hi
    rhsv[49, :] = (cck - cck_hi.astype(f64)).astype(np.float16)

    const0 = (math.lgamma(float(K)) + (K - 1) * math.log(TAU)
              + float(lnpi64.sum()))
    # (z quantization bias is handled by the sampled correction instead)
    ctx = {"a": a, "mu2": mu2, "ck": ck, "pisoft": pisoft}
    return rhsv, pisoft, lnpi64, const0, ctx


def _host_small_losses(mu, pi, lambda_mu, b, C, r, lnpi64, R):
    """All parameter-only losses in float64, mirroring the reference."""
    f64 = np.float64
    Df = float(D)
    c = 1.25 + (D - 1) / 4.0
    g = 0.25 + (D - 1) / 4.0
    G = c / (50.0 * g) * math.sqrt(float((R ** 2).sum()))

    pi_loss = -((1.0 / K - 1.0) * lnpi64).sum()

    lam = lambda_mu.astype(f64)
    var_mu = (lam ** 2) * R
    mu64 = mu.astype(f64)
    b64 = b.astype(f64)
    mu_lp = (-0.5 * (((mu64 - b64) ** 2) / var_mu[None, :]).sum(1)
             - 0.5 * np.log(var_mu).sum() - 0.5 * Df * LOG2PI)
    mu_loss = -mu_lp.sum()

    lam_lp = (0.5 * math.log(0.5) - math.lgamma(0.5)
              + (0.5 - 1.0) * lam - 0.5 * np.exp(lam))
    lambda_loss = -lam_lp.sum()

    b_loss = 0.5 * (b64 ** 2).sum() + 0.5 * K * Df * LOG2PI

    r64 = r.astype(f64)
    C64 = C.astype(f64)
    r_lp = (c * np.log(C64) + (c - 1.0) * (-r64) - C64 * np.exp(-r64)
            - math.lgamma(c))
    r_loss = -r_lp.sum()

    C_lp = (g * math.log(G) + (g - 1.0) * (-C64) - G * np.exp(-C64)
            - math.lgamma(g))
    C_loss = -C_lp.sum()

    return r_loss + mu_loss + pi_loss + b_loss + lambda_loss + C_loss


def kernel(met_locs, mu, pi, lambda_mu, b, C, r, z):
    met_locs = np.asarray(met_locs, dtype=np.float32)
    mu = np.asarray(mu, dtype=np.float32)
    pi = np.asarray(pi, dtype=np.float32)
    lambda_mu = np.asarray(lambda_mu, dtype=np.float32)
    b = np.asarray(b, dtype=np.float32)
    C = np.asarray(C, dtype=np.float32)
    r = np.asarray(r, dtype=np.float32)
    z = np.asarray(z, dtype=np.float32)

    CHZ, CHM = 2048, 8192
    if "run" not in _cache:
        _cache["nc"] = _build_program()
        _cache["run"] = _make_runner(_cache["nc"])
        _cache["zcb"] = np.empty((CHZ, K), np.float32)
        _cache["zcu"] = np.empty((CHZ, K), np.uint8)
        _cache["zct"] = np.empty((CHZ // 256, 128, 64), np.uint8)
        _cache["u8s"] = np.empty((MSAMP, K), np.uint8)
        _cache["zq4a"] = np.empty((N // 4, 64), np.uint8)
        _cache["zq4b"] = np.empty((N // 4, 64), np.uint8)
        _cache["mcb"] = np.empty((CHM, D), np.float32)
        _cache["mqi"] = np.empty((N, D), np.int8)
        _cache["mqg"] = np.empty((NCORES * 16, NS), np.int8)
        _cache["rhsvg"] = np.empty((NCORES * NRHS, 64), np.float16)
        _cache["piveg"] = np.empty((NCORES * 128, 64), np.float32)
        _cache["mscg"] = np.empty((NCORES * 16, 1), np.float32)
    run = _cache["run"]

    # ---- z: int4, fixed scale, biased codes u = rint(z/L4)+8 in [1,15];
    # rows r and r+128 of each 256-row chunk share a byte (lo|hi<<4), so
    # both nibbles of a byte sit on the same SBUF partition on device.
    # Chunked so the f32 intermediate stays in cache (single DRAM sweep).
    zcb, zcu, zct = _cache["zcb"], _cache["zcu"], _cache["zct"]
    u8s = _cache["u8s"]
    zq4a_g, zq4b_g = _cache["zq4a"], _cache["zq4b"]
    za_v = zq4a_g.reshape(N // 512, 128, 64)   # [core*64 + s8*8 + b]
    zb_v = zq4b_g.reshape(N // 512, 128, 64)
    zstep = N // MSAMP
    spc = CHZ // zstep                         # sampled rows per chunk
    for i in range(0, N, CHZ):
        np.multiply(z[i:i + CHZ], np.float32(1.0 / L4), out=zcb)
        np.add(zcb, np.float32(8.5), out=zcb)
        np.copyto(zcu, zcb, casting="unsafe")  # trunc == floor (all > 0)
        j = i // zstep
        u8s[j:j + spc] = zcu[::zstep]          # keep for sampled correction
        v = zcu.reshape(CHZ // 256, 2, 128, 64)
        np.left_shift(v[:, 1], 4, out=zct)
        core, sc = divmod(i // CHZ, N_SC)
        half, s8 = divmod(sc, N_SC // 2)
        dst = (za_v if half == 0 else zb_v)
        b0 = core * 64 + s8 * 8
        np.bitwise_or(v[:, 0], zct, out=dst[b0:b0 + 8])

    # ---- met: int8 transposed, per-dim scales from a sampled absmax
    # estimate (headroom 1.18x + clip makes overflow impossible); the exact
    # per-dim max/min for R is computed after dispatch, under the stream.
    mmax = np.abs(met_locs[::16]).max(0).astype(np.float64) * 1.18
    mmax = np.maximum(mmax, 1e-12)
    msf = (127.0 / mmax).astype(np.float32)            # [16]
    mcb, mqi = _cache["mcb"], _cache["mqi"]
    for i in range(0, N, CHM):
        np.multiply(met_locs[i:i + CHM], msf[None, :], out=mcb)
        np.rint(mcb, out=mcb)
        np.minimum(mcb, np.float32(127.0), out=mcb)
        np.maximum(mcb, np.float32(-127.0), out=mcb)
        np.copyto(mqi[i:i + CHM], mcb, casting="unsafe")
    mq_g = _cache["mqg"]
    for c in range(NCORES):
        mq_g[c * 16:(c + 1) * 16] = mqi[c * NS:(c + 1) * NS].T

    met_l2 = float(((mmax / 127.0) ** 2).sum())
    rhsv, pisoft, lnpi64, const0, ctx = _prep_consts(mu, pi, r, met_l2)

    rhsv_g, pive_g, msc_g = _cache["rhsvg"], _cache["piveg"], _cache["mscg"]
    rhsv_g.reshape(NCORES, NRHS, 64)[:] = rhsv[None]
    pive_g.reshape(NCORES, 128, 64)[:] = pisoft.astype(np.float32)[None, None]
    msc_g.reshape(NCORES, 16)[:] = (mmax / 127.0).astype(np.float32)[None]

    globals_map = {
        "mq": mq_g,
        "zq4a": zq4a_g,
        "zq4b": zq4b_g,
        "rhsv": rhsv_g,
        "pivec": pive_g,
        "mscale": msc_g,
    }

    outs = run(globals_map)                       # async dispatch

    # ---- overlapped with the transfer: exact per-dim range for R, small
    # losses, and the exact sampled int4-bias correction (f64)
    mx = np.full(D, -np.inf, np.float32)
    mn = np.full(D, np.inf, np.float32)
    for i in range(0, N, CHM):
        c = met_locs[i:i + CHM]
        np.maximum(mx, c.max(0), out=mx)
        np.minimum(mn, c.min(0), out=mn)
    R = (mx.astype(np.float64) - mn.astype(np.float64))
    small = _host_small_losses(mu, pi, lambda_mu, b, C, r, lnpi64, R)

    f64 = np.float64
    idx = slice(0, N, N // MSAMP)
    z_s = z[idx].astype(f64)
    zdq_s = (u8s.astype(f64) - 8.0) * L4
    x_s = met_locs[idx].astype(f64)
    a64, mu264, ck64 = ctx["a"], ctx["mu2"], ctx["ck"]
    pis64 = ctx["pisoft"]
    mu64 = mu.astype(f64)
    x2_s = (x_s ** 2).sum(1)
    logN_s = (a64[None, :] * (x2_s[:, None] - 2.0 * (x_s @ mu64.T)
                              + mu264[None, :]) + ck64[None, :])

    def _stat(zz):
        v = zz + logN_s
        mxv = v.max(1)
        su = np.exp(v - mxv[:, None]).sum(1)
        sz = np.exp(zz).sum(1)
        st = (pis64[None, :] * np.exp(-TAU * zz)).sum(1)
        return (mxv + np.log(su) + 63.0 * np.log(sz)
                - 64.0 * np.log(st) - 1.1 * zz.sum(1))

    corr = float((_stat(z_s) - _stat(zdq_s)).mean()) * N

    dev_sum = np.asarray(outs["out"]).astype(np.float64).sum()
    z_loss = -(dev_sum + corr + N * const0)
    total = z_loss + small
    return np.asarray(total, dtype=np.float32)


# revision 40
# speedup vs baseline: 1.0658x; 1.0658x over previous
"""Trainium2 Bass kernel for nn_Clusterer loss (Concrete-mixture clustering loss).

Strategy (data-parallel over N across 8 cores, per sharding hint):
  - Minimal wire traffic through the PJRT tunnel (~12.3MB vs 80MB raw fp32):
      z   -> int4 (two values per byte, rows 128 apart share a byte so both
             nibbles land on the same SBUF partition), fixed scale, NATURAL
             row layout; unpacked on device with DVE bitwise and/shift and
             dequantized by ACT Copy (scale=L, bias=-8L)
      met -> int8 transposed [16, NS], per-dim adaptive scales
    The int4 quantization bias of the ln-sum terms is removed by an exact
    sampled correction: stat(z) - stat(dequant(z)) on a deterministic strided
    row subsample in f64 on the host, scaled to N - computed while the
    transfer streams, so it costs no wall time.
  - logN per 128-row tile by four accumulated fp16 matmuls:
      mm1: met16 x w;  mm2/mm3: xsq(=ACT Square of met16) x a_hi / a_lo;
      mm4: ones(2) x [cck_hi; cck_lo]
    (x2 is built on device; hi/lo split of a_k kills fp16 systematic error;
     int8 quantization E[d^2] bias is compensated in cck / const0 on host.)
  - v = logN + z; row logsumexp on DVE/ACT.  Concrete-prior row sums
    (sum e^z, sum pi e^{-tau z}, sum z) are free-axis reductions on the
    natural-layout z tile - z is shipped exactly once, never transposed.
  - Per-row total = max_v + ln su + 63 ln sz - 64 ln st - 1.1 s1 summed on
    device to [128, 1] per core; final f64 reduction + tiny parameter
    losses on host (overlapped with the device call).
  - Dispatch through a cached jit(shard_map(...)) built once per process:
    no per-call retracing, no concat copies (int8 z global array IS the
    axis-0-sharded layout).
"""

import math

import numpy as np

N, D, K = 262144, 16, 64
NCORES = 8
NS = N // NCORES          # rows per core = 32768
NG = NS // 128            # 128-row groups per core = 256
G_SC = 16                 # groups per super-chunk
N_SC = NG // G_SC         # super-chunks = 16
FD_SC = G_SC * 64         # free dim per SC = 1024
TAU = 0.1
LOG2PI = math.log(2.0 * math.pi)
ZLIM = 6.8                # fixed |z| quantization range
L4 = ZLIM / 7.49          # int4 step: codes rint(z/L4) in [-7, 7], biased +8
MSAMP = 8192              # rows in the sampled quantization-bias correction
NRHS = 50                 # rhs rows: 16 w + 16 a_hi + 16 a_lo + 2 cck hi/lo

_cache = {}

_QUANT_C = r"""
#include <stdint.h>
void quant4(const float* z, uint8_t* za, uint8_t* zb, uint8_t* u8s,
            float s, long nblk) {
    for (long b = 0; b < nblk; b++) {          /* 128-packed-row blocks */
        long jj = b / 8, b8 = b % 8;           /* 2048-row chunk, block */
        long core = jj / 16, sc = jj % 16;
        uint8_t* dst = (sc < 8 ? za : zb)
                     + ((core * 64 + (sc % 8) * 8 + b8) << 13);
        const float* zlo = z + b * 256 * 64;
        const float* zhi = zlo + 128 * 64;
        for (long p = 0; p < 128; p++) {
            const float* rl = zlo + p * 64;
            const float* rh = zhi + p * 64;
            uint8_t* d = dst + p * 64;
            for (long k = 0; k < 64; k++) {
                uint8_t lo = (uint8_t)(rl[k] * s + 8.5f);
                uint8_t hi = (uint8_t)(rh[k] * s + 8.5f);
                d[k] = (uint8_t)(lo | (hi << 4));
            }
            long glo = b * 256 + p;
            if ((glo & 31) == 0) {
                uint8_t* u = u8s + (glo >> 5) * 64;
                for (long k = 0; k < 64; k++)
                    u[k] = (uint8_t)(rl[k] * s + 8.5f);
            }
            if (((glo + 128) & 31) == 0) {
                uint8_t* u = u8s + ((glo + 128) >> 5) * 64;
                for (long k = 0; k < 64; k++)
                    u[k] = (uint8_t)(rh[k] * s + 8.5f);
            }
        }
    }
}
"""


def _build_cquant():
    """Compile the fused quant+pack kernel; return ctypes fn or None."""
    import ctypes
    import subprocess
    import tempfile

    try:
        d = tempfile.mkdtemp(prefix="q4c_")
        src = f"{d}/q4.c"
        so = f"{d}/q4.so"
        with open(src, "w") as f:
            f.write(_QUANT_C)
        subprocess.run(
            ["gcc", "-O3", "-march=native", "-ffast-math", "-shared",
             "-fPIC", "-o", so, src],
            check=True, capture_output=True, timeout=60)
        lib = ctypes.CDLL(so)
        lib.quant4.argtypes = [
            ctypes.c_void_p, ctypes.c_void_p, ctypes.c_void_p,
            ctypes.c_void_p, ctypes.c_float, ctypes.c_long]
        return lib.quant4
    except Exception:
        return None


def _build_program():
    import concourse.bacc as bacc
    import concourse.mybir as mybir
    import concourse.tile as tile

    fp16 = mybir.dt.float16
    fp32 = mybir.dt.float32
    int8 = mybir.dt.int8
    uint8 = mybir.dt.uint8
    AF = mybir.ActivationFunctionType
    ALU = mybir.AluOpType
    AX = mybir.AxisListType

    nc = bacc.Bacc("TRN2", target_bir_lowering=False, debug=False,
                   num_devices=NCORES)

    mq = nc.dram_tensor("mq", [16, NS], int8, kind="ExternalInput").ap()
    # packed z split in two args: staging of the later args pipelines behind
    # the wire streaming of the earlier ones (finer args measured faster)
    zq4a = nc.dram_tensor("zq4a", [NS // 4, 64], uint8,
                          kind="ExternalInput").ap()
    zq4b = nc.dram_tensor("zq4b", [NS // 4, 64], uint8,
                          kind="ExternalInput").ap()
    rhsv = nc.dram_tensor("rhsv", [NRHS, 64], fp16, kind="ExternalInput").ap()
    pivec = nc.dram_tensor("pivec", [128, 64], fp32, kind="ExternalInput").ap()
    mscale = nc.dram_tensor("mscale", [16, 1], fp32, kind="ExternalInput").ap()
    out = nc.dram_tensor("out", [128, 1], fp32, kind="ExternalOutput").ap()

    # packed z: byte (s, c, p, k) holds z rows (s*16 + 2c)*128+p (lo nibble)
    # and (s*16 + 2c+1)*128+p (hi nibble); view as [p, s, c, k]
    zq_ra = zq4a.rearrange("(s c p) k -> p s c k",
                           s=N_SC // 2, c=G_SC // 2, p=128)
    zq_rb = zq4b.rearrange("(s c p) k -> p s c k",
                           s=N_SC // 2, c=G_SC // 2, p=128)

    with tile.TileContext(nc) as tc:
        with (
            tc.tile_pool(name="const", bufs=1) as constp,
            tc.tile_pool(name="stats", bufs=1) as statp,
            tc.tile_pool(name="mq", bufs=3) as mqp,
            tc.tile_pool(name="pkd", bufs=2) as pkdp,
            tc.tile_pool(name="xsq", bufs=2) as xsqp,
            tc.tile_pool(name="zqp", bufs=3) as zqp,
            tc.tile_pool(name="z16p", bufs=2) as z16p,
            tc.tile_pool(name="vp", bufs=2) as vp,
            tc.tile_pool(name="scr", bufs=2) as scrp,
            tc.tile_pool(name="ep", bufs=1) as epp,
            tc.tile_pool(name="ps", bufs=2, space="PSUM") as psp,
        ):
            rhsA = constp.tile([16, 64], fp16, tag="rhsA")
            nc.sync.dma_start(rhsA[:], rhsv[0:16, :])
            rhsB = constp.tile([16, 64], fp16, tag="rhsB")
            nc.sync.dma_start(rhsB[:], rhsv[16:32, :])
            rhsC = constp.tile([16, 64], fp16, tag="rhsC")
            nc.sync.dma_start(rhsC[:], rhsv[32:48, :])
            rhsD = constp.tile([2, 64], fp16, tag="rhsD")
            nc.sync.dma_start(rhsD[:], rhsv[48:50, :])
            ones2 = constp.tile([2, 128], fp16, tag="ones2")
            nc.vector.memset(ones2[:], 1.0)
            pi_s = constp.tile([128, 64], fp32, tag="pis")
            nc.sync.dma_start(pi_s[:], pivec[:])
            msc = constp.tile([16, 1], fp32, tag="msc")
            nc.sync.dma_start(msc[:], mscale[:])
            pi_t = constp.tile([128, FD_SC], fp32, tag="pit")
            for i in range(G_SC):
                nc.vector.tensor_copy(pi_t[:, i * 64:(i + 1) * 64], pi_s[:])

            mu_all = statp.tile([128, NG], fp32, tag="mu_all")
            su_all = statp.tile([128, NG], fp32, tag="su_all")
            sz_all = statp.tile([128, NG], fp32, tag="sz_all")
            st_all = statp.tile([128, NG], fp32, tag="st_all")
            s1_all = statp.tile([128, NG], fp32, tag="s1_all")

            for sc in range(N_SC):
                sl = slice(sc * G_SC, (sc + 1) * G_SC)
                fsl = slice(sc * G_SC * 128, (sc + 1) * G_SC * 128)

                mq_t = mqp.tile([16, G_SC * 128], int8, tag="mq")
                nc.sync.dma_start(mq_t[:], mq[:, fsl])
                pkd = pkdp.tile([16, G_SC * 128], fp16, tag="pkd")
                nc.scalar.activation(pkd[:], mq_t[:], AF.Copy,
                                     scale=msc[:, 0:1])
                xsq = xsqp.tile([16, G_SC * 128], fp16, tag="xsq")
                nc.scalar.activation(xsq[:], pkd[:], AF.Square)

                zp_t = zqp.tile([128, FD_SC // 2], uint8, tag="zp")
                zq_r = zq_ra if sc < N_SC // 2 else zq_rb
                sch = sc % (N_SC // 2)
                nc.sync.dma_start(
                    zp_t[:].rearrange("p (o c k) -> p o c k", o=1, k=64),
                    zq_r[:, sch:sch + 1, :, :])
                ulo = zqp.tile([128, FD_SC // 2], uint8, tag="ulo")
                nc.vector.tensor_scalar(ulo[:], zp_t[:], 15, None,
                                        op0=ALU.bitwise_and)
                uhi = zqp.tile([128, FD_SC // 2], uint8, tag="uhi")
                nc.vector.tensor_scalar(uhi[:], zp_t[:], 4, None,
                                        op0=ALU.logical_shift_right)
                z16 = z16p.tile([128, FD_SC], fp16, tag="z16")
                z16v = z16[:].rearrange("p (c t k) -> p c t k", t=2, k=64)
                nc.scalar.activation(
                    z16v[:, :, 0, :],
                    ulo[:].rearrange("p (c k) -> p c k", k=64),
                    AF.Copy, scale=L4, bias=-8.0 * L4)
                nc.scalar.activation(
                    z16v[:, :, 1, :],
                    uhi[:].rearrange("p (c k) -> p c k", k=64),
                    AF.Copy, scale=L4, bias=-8.0 * L4)

                ps = psp.tile([128, FD_SC], fp32, tag="v")
                for g in range(G_SC):
                    gs = slice(g * 128, (g + 1) * 128)
                    nc.tensor.matmul(ps[:, g * 64:(g + 1) * 64],
                                     lhsT=pkd[:, gs], rhs=rhsA[:],
                                     start=True, stop=False)
                    nc.tensor.matmul(ps[:, g * 64:(g + 1) * 64],
                                     lhsT=xsq[:, gs], rhs=rhsB[:],
                                     start=False, stop=False)
                    nc.tensor.matmul(ps[:, g * 64:(g + 1) * 64],
                                     lhsT=xsq[:, gs], rhs=rhsC[:],
                                     start=False, stop=False)
                    nc.tensor.matmul(ps[:, g * 64:(g + 1) * 64],
                                     lhsT=ones2[:], rhs=rhsD[:],
                                     start=False, stop=True)

                v = vp.tile([128, FD_SC], fp32, tag="vv")
                nc.vector.tensor_add(v[:], ps[:], z16[:])
                v3 = v[:].rearrange("p (g k) -> p g k", k=64)
                mu_sl = mu_all[:, sl]
                nc.vector.reduce_max(mu_sl, v3, axis=AX.X)
                vs = scrp.tile([128, FD_SC], fp32, tag="vs")
                mu_b = mu_sl.broadcast_to([128, G_SC, 64])
                nc.vector.scalar_tensor_tensor(
                    vs[:].rearrange("p (g k) -> p g k", k=64),
                    in0=v3, scalar=1.0, in1=mu_b,
                    op0=ALU.mult, op1=ALU.subtract)
                eu = scrp.tile([128, FD_SC], fp16, tag="eu")
                nc.scalar.activation(eu[:], vs[:], AF.Exp)
                nc.vector.reduce_sum(
                    su_all[:, sl],
                    eu[:].rearrange("p (g k) -> p g k", k=64), axis=AX.X)

                e1 = scrp.tile([128, FD_SC], fp16, tag="e1")
                nc.scalar.activation(e1[:], z16[:], AF.Exp)
                nc.vector.reduce_sum(
                    sz_all[:, sl],
                    e1[:].rearrange("p (g k) -> p g k", k=64), axis=AX.X)

                e2 = scrp.tile([128, FD_SC], fp16, tag="e2")
                nc.scalar.activation(e2[:], z16[:], AF.Exp, scale=-TAU)
                w2 = scrp.tile([128, FD_SC], fp32, tag="w2")
                nc.vector.tensor_mul(w2[:], e2[:], pi_t[:])
                nc.vector.reduce_sum(
                    st_all[:, sl],
                    w2[:].rearrange("p (g k) -> p g k", k=64), axis=AX.X)

                nc.vector.reduce_sum(
                    s1_all[:, sl],
                    z16[:].rearrange("p (g k) -> p g k", k=64), axis=AX.X)

            # ---- epilogue: mu + ln su + 63 ln sz - 64 ln st - 1.1 s1 ----
            lnsu = epp.tile([128, NG], fp32, tag="lnsu")
            nc.scalar.activation(lnsu[:], su_all[:], AF.Ln)
            lnsz = epp.tile([128, NG], fp32, tag="lnsz")
            nc.scalar.activation(lnsz[:], sz_all[:], AF.Ln)
            lnst = epp.tile([128, NG], fp32, tag="lnst")
            nc.scalar.activation(lnst[:], st_all[:], AF.Ln)
            acc = epp.tile([128, NG], fp32, tag="acc")
            nc.vector.tensor_add(acc[:], mu_all[:], lnsu[:])
            acc2 = epp.tile([128, NG], fp32, tag="acc2")
            nc.vector.scalar_tensor_tensor(
                acc2[:], in0=lnsz[:], scalar=63.0, in1=acc[:],
                op0=ALU.mult, op1=ALU.add)
            acc3 = epp.tile([128, NG], fp32, tag="acc3")
            nc.vector.scalar_tensor_tensor(
                acc3[:], in0=lnst[:], scalar=-64.0, in1=acc2[:],
                op0=ALU.mult, op1=ALU.add)
            acc4 = epp.tile([128, NG], fp32, tag="acc4")
            nc.vector.scalar_tensor_tensor(
                acc4[:], in0=s1_all[:], scalar=-1.1, in1=acc3[:],
                op0=ALU.mult, op1=ALU.add)
            out_t = epp.tile([128, 1], fp32, tag="outt")
            nc.vector.reduce_sum(out_t[:], acc4[:], axis=AX.X)
            nc.sync.dma_start(out[:], out_t[:])

    nc.compile()
    return nc


def _make_runner(nc):
    """Cached jit(shard_map(...)) dispatcher; built once, reused every call."""
    import jax
    from jax.experimental.shard_map import shard_map
    from jax.sharding import Mesh, PartitionSpec

    import concourse.mybir as mybir
    from concourse.bass2jax import (_bass_exec_p, install_neuronx_cc_hook,
                                    partition_id_tensor)

    install_neuronx_cc_hook()

    partition_name = (nc.partition_id_tensor.name
                      if nc.partition_id_tensor else None)
    in_names, out_names, out_avals, zero_shapes = [], [], [], []
    for alloc in nc.m.functions[0].allocations:
        if not isinstance(alloc, mybir.MemoryLocationSet):
            continue
        name = alloc.memorylocations[0].name
        if alloc.kind == "ExternalInput":
            if name != partition_name:
                in_names.append(name)
        elif alloc.kind == "ExternalOutput":
            shape = tuple(alloc.tensor_shape)
            dtype = mybir.dt.np(alloc.dtype)
            out_names.append(name)
            out_avals.append(jax.core.ShapedArray(shape, dtype))
            zero_shapes.append((shape, dtype))
    n_params = len(in_names)
    all_names = list(in_names) + list(out_names)
    if partition_name is not None:
        all_names.append(partition_name)
    donate = tuple(range(n_params, n_params + len(out_names)))

    def _body(*args):
        operands = list(args)
        if partition_name is not None:
            operands.append(partition_id_tensor())
        outs = _bass_exec_p.bind(
            *operands,
            out_avals=tuple(out_avals),
            in_names=tuple(all_names),
            out_names=tuple(out_names),
            lowering_input_output_aliases=(),
            sim_require_finite=True,
            sim_require_nnan=True,
            nc=nc,
        )
        return tuple(outs)

    devices = jax.devices()[:NCORES]
    assert len(devices) == NCORES
    mesh = Mesh(np.asarray(devices), ("core",))
    in_specs = (PartitionSpec("core"),) * (n_params + len(out_names))
    out_specs = (PartitionSpec("core"),) * len(out_names)
    sharded = jax.jit(
        shard_map(_body, mesh=mesh, in_specs=in_specs, out_specs=out_specs,
                  check_rep=False),
        donate_argnums=donate, keep_unused=True)

    def run(globals_map):
        """Dispatch; returns unforced jax outputs keyed by name."""
        ins = [globals_map[n] for n in in_names]
        zeros = [np.zeros((NCORES * s[0], *s[1:]), d) for s, d in zero_shapes]
        outs = sharded(*ins, *zeros)
        return {name: outs[i] for i, name in enumerate(out_names)}

    return run


def _prep_consts(mu, pi, r, met_l2):
    """K-sized constants in f64 -> fp16 hi/lo packed rhs + pi softmax.

    met_l2 = sum_f LSB_f^2 of the met quantizer (for E[d^2] bias comp).
    """
    f64 = np.float64
    mu64 = mu.astype(f64)
    r64 = r.astype(f64)
    pi64 = pi.astype(f64)

    a = -0.5 * np.exp(-r64)                       # [K]
    mu2 = (mu64 ** 2).sum(1)                      # [K]
    ck = -0.5 * D * (r64 + LOG2PI)                # [K]
    cck = a * mu2 + ck - a * (met_l2 / 12.0)      # [K], with quant-bias comp
    m = pi64.max()
    lnpi64 = pi64 - (m + np.log(np.exp(pi64 - m).sum()))
    pisoft = np.exp(lnpi64)

    rhsv = np.zeros((NRHS, 64), np.float16)
    rhsv[0:16, :] = (-2.0 * a[None, :] * mu64.T).astype(np.float16)
    a_hi = a.astype(np.float16)
    a_lo = (a - a_hi.astype(f64)).astype(np.float16)
    rhsv[16:32, :] = a_hi[None, :]
    rhsv[32:48, :] = a_lo[None, :]
    cck_hi = cck.astype(np.float16)
    rhsv[48, :] = cck_hi
    rhsv[49, :] = (cck - cck_hi.astype(f64)).astype(np.float16)

    const0 = (math.lgamma(float(K)) + (K - 1) * math.log(TAU)
              + float(lnpi64.sum()))
    # (z quantization bias is handled by the sampled correction instead)
    ctx = {"a": a, "mu2": mu2, "ck": ck, "pisoft": pisoft}
    return rhsv, pisoft, lnpi64, const0, ctx


def _host_small_losses(mu, pi, lambda_mu, b, C, r, lnpi64, R):
    """All parameter-only losses in float64, mirroring the reference."""
    f64 = np.float64
    Df = float(D)
    c = 1.25 + (D - 1) / 4.0
    g = 0.25 + (D - 1) / 4.0
    G = c / (50.0 * g) * math.sqrt(float((R ** 2).sum()))

    pi_loss = -((1.0 / K - 1.0) * lnpi64).sum()

    lam = lambda_mu.astype(f64)
    var_mu = (lam ** 2) * R
    mu64 = mu.astype(f64)
    b64 = b.astype(f64)
    mu_lp = (-0.5 * (((mu64 - b64) ** 2) / var_mu[None, :]).sum(1)
             - 0.5 * np.log(var_mu).sum() - 0.5 * Df * LOG2PI)
    mu_loss = -mu_lp.sum()

    lam_lp = (0.5 * math.log(0.5) - math.lgamma(0.5)
              + (0.5 - 1.0) * lam - 0.5 * np.exp(lam))
    lambda_loss = -lam_lp.sum()

    b_loss = 0.5 * (b64 ** 2).sum() + 0.5 * K * Df * LOG2PI

    r64 = r.astype(f64)
    C64 = C.astype(f64)
    r_lp = (c * np.log(C64) + (c - 1.0) * (-r64) - C64 * np.exp(-r64)
            - math.lgamma(c))
    r_loss = -r_lp.sum()

    C_lp = (g * math.log(G) + (g - 1.0) * (-C64) - G * np.exp(-C64)
            - math.lgamma(g))
    C_loss = -C_lp.sum()

    return r_loss + mu_loss + pi_loss + b_loss + lambda_loss + C_loss


def kernel(met_locs, mu, pi, lambda_mu, b, C, r, z):
    met_locs = np.asarray(met_locs, dtype=np.float32)
    mu = np.asarray(mu, dtype=np.float32)
    pi = np.asarray(pi, dtype=np.float32)
    lambda_mu = np.asarray(lambda_mu, dtype=np.float32)
    b = np.asarray(b, dtype=np.float32)
    C = np.asarray(C, dtype=np.float32)
    r = np.asarray(r, dtype=np.float32)
    z = np.asarray(z, dtype=np.float32)

    CHZ, CHM = 2048, 8192
    if "run" not in _cache:
        _cache["nc"] = _build_program()
        _cache["run"] = _make_runner(_cache["nc"])
        _cache["zcb"] = np.empty((CHZ, K), np.float32)
        _cache["zcu"] = np.empty((CHZ, K), np.uint8)
        _cache["zct"] = np.empty((CHZ // 256, 128, 64), np.uint8)
        _cache["u8s"] = np.empty((MSAMP, K), np.uint8)
        _cache["zq4a"] = np.empty((N // 4, 64), np.uint8)
        _cache["zq4b"] = np.empty((N // 4, 64), np.uint8)
        _cache["mcb"] = np.empty((CHM, D), np.float32)
        _cache["mqi"] = np.empty((N, D), np.int8)
        _cache["mqg"] = np.empty((NCORES * 16, NS), np.int8)
        _cache["cq"] = _build_cquant()
        _cache["rhsvg"] = np.empty((NCORES * NRHS, 64), np.float16)
        _cache["piveg"] = np.empty((NCORES * 128, 64), np.float32)
        _cache["mscg"] = np.empty((NCORES * 16, 1), np.float32)
    run = _cache["run"]

    # ---- z: int4, fixed scale, biased codes u = rint(z/L4)+8 in [1,15];
    # rows r and r+128 of each 256-row chunk share a byte (lo|hi<<4), so
    # both nibbles of a byte sit on the same SBUF partition on device.
    # Chunked so the f32 intermediate stays in cache (single DRAM sweep).
    zcb, zcu, zct = _cache["zcb"], _cache["zcu"], _cache["zct"]
    u8s = _cache["u8s"]
    zq4a_g, zq4b_g = _cache["zq4a"], _cache["zq4b"]
    za_v = zq4a_g.reshape(N // 512, 128, 64)   # [core*64 + s8*8 + b]
    zb_v = zq4b_g.reshape(N // 512, 128, 64)
    cq = _cache["cq"]
    zc = np.ascontiguousarray(z) if not z.flags.c_contiguous else z
    if cq is not None:
        import ctypes
        cq(zc.ctypes.data_as(ctypes.c_void_p),
           zq4a_g.ctypes.data_as(ctypes.c_void_p),
           zq4b_g.ctypes.data_as(ctypes.c_void_p),
           u8s.ctypes.data_as(ctypes.c_void_p),
           ctypes.c_float(1.0 / L4), N // 256)
    else:
        zstep = N // MSAMP
        spc = CHZ // zstep                     # sampled rows per chunk
        for i in range(0, N, CHZ):
            np.multiply(zc[i:i + CHZ], np.float32(1.0 / L4), out=zcb)
            np.add(zcb, np.float32(8.5), out=zcb)
            np.copyto(zcu, zcb, casting="unsafe")  # trunc == floor (all > 0)
            j = i // zstep
            u8s[j:j + spc] = zcu[::zstep]      # keep for sampled correction
            v = zcu.reshape(CHZ // 256, 2, 128, 64)
            np.left_shift(v[:, 1], 4, out=zct)
            core, sc = divmod(i // CHZ, N_SC)
            half, s8 = divmod(sc, N_SC // 2)
            dst = (za_v if half == 0 else zb_v)
            b0 = core * 64 + s8 * 8
            np.bitwise_or(v[:, 0], zct, out=dst[b0:b0 + 8])

    # ---- met: int8 transposed, per-dim scales from a sampled absmax
    # estimate (headroom 1.18x + clip makes overflow impossible); the exact
    # per-dim max/min for R is computed after dispatch, under the stream.
    mmax = np.abs(met_locs[::16]).max(0).astype(np.float64) * 1.18
    mmax = np.maximum(mmax, 1e-12)
    msf = (127.0 / mmax).astype(np.float32)            # [16]
    mcb, mqi = _cache["mcb"], _cache["mqi"]
    for i in range(0, N, CHM):
        np.multiply(met_locs[i:i + CHM], msf[None, :], out=mcb)
        np.rint(mcb, out=mcb)
        np.minimum(mcb, np.float32(127.0), out=mcb)
        np.maximum(mcb, np.float32(-127.0), out=mcb)
        np.copyto(mqi[i:i + CHM], mcb, casting="unsafe")
    mq_g = _cache["mqg"]
    for c in range(NCORES):
        mq_g[c * 16:(c + 1) * 16] = mqi[c * NS:(c + 1) * NS].T

    met_l2 = float(((mmax / 127.0) ** 2).sum())
    rhsv, pisoft, lnpi64, const0, ctx = _prep_consts(mu, pi, r, met_l2)

    rhsv_g, pive_g, msc_g = _cache["rhsvg"], _cache["piveg"], _cache["mscg"]
    rhsv_g.reshape(NCORES, NRHS, 64)[:] = rhsv[None]
    pive_g.reshape(NCORES, 128, 64)[:] = pisoft.astype(np.float32)[None, None]
    msc_g.reshape(NCORES, 16)[:] = (mmax / 127.0).astype(np.float32)[None]

    globals_map = {
        "mq": mq_g,
        "zq4a": zq4a_g,
        "zq4b": zq4b_g,
        "rhsv": rhsv_g,
        "pivec": pive_g,
        "mscale": msc_g,
    }

    outs = run(globals_map)                       # async dispatch

    # ---- overlapped with the transfer: exact per-dim range for R, small
    # losses, and the exact sampled int4-bias correction (f64)
    mx = np.full(D, -np.inf, np.float32)
    mn = np.full(D, np.inf, np.float32)
    for i in range(0, N, CHM):
        c = met_locs[i:i + CHM]
        np.maximum(mx, c.max(0), out=mx)
        np.minimum(mn, c.min(0), out=mn)
    R = (mx.astype(np.float64) - mn.astype(np.float64))
    small = _host_small_losses(mu, pi, lambda_mu, b, C, r, lnpi64, R)

    f64 = np.float64
    idx = slice(0, N, N // MSAMP)
    z_s = z[idx].astype(f64)
    zdq_s = (u8s.astype(f64) - 8.0) * L4
    x_s = met_locs[idx].astype(f64)
    a64, mu264, ck64 = ctx["a"], ctx["mu2"], ctx["ck"]
    pis64 = ctx["pisoft"]
    mu64 = mu.astype(f64)
    x2_s = (x_s ** 2).sum(1)
    logN_s = (a64[None, :] * (x2_s[:, None] - 2.0 * (x_s @ mu64.T)
                              + mu264[None, :]) + ck64[None, :])

    def _stat(zz):
        v = zz + logN_s
        mxv = v.max(1)
        su = np.exp(v - mxv[:, None]).sum(1)
        sz = np.exp(zz).sum(1)
        st = (pis64[None, :] * np.exp(-TAU * zz)).sum(1)
        return (mxv + np.log(su) + 63.0 * np.log(sz)
                - 64.0 * np.log(st) - 1.1 * zz.sum(1))

    corr = float((_stat(z_s) - _stat(zdq_s)).mean()) * N

    dev_sum = np.asarray(outs["out"]).astype(np.float64).sum()
    z_loss = -(dev_sum + corr + N * const0)
    total = z_loss + small
    return np.asarray(total, dtype=np.float32)
